# revision 13
# baseline (speedup 1.0000x reference)
"""MoE (8 experts, top-2, H=2048, N=4096 tokens) Trainium2 kernel.

Strategy (expert-parallel over 8 NeuronCores):
  - Gate (softmax + top-2) computed host-side with jax-CPU fp32 so the
    routing decisions bit-match the jax reference.
  - Each core c handles expert e=c: tokens routed to e are gathered,
    transposed to X^T [H, C] (C = padded capacity), cast to bf16.
  - On-device per core: hT = relu(W1T.T @ XT [+ b1]); yT = second matmul.
    Weights stay stationary; both matmuls emit transposed activations so
    no on-device transpose is needed. bf16 inputs, fp32 PSUM accumulate.
  - b1 (if nonzero) enters as a rank-1 matmul into the psum group;
    b2 (if nonzero) is added host-side.
  - Host scatters y back: out[idx_e] += w_e[:,None] * y_e.

Hardware constraint honored throughout: each compute instruction carries at
most ONE semaphore wait, so no instruction may depend on two distinct
processors (e.g. PSUM-ready + DMA-loaded); staging tiles are never reused.
"""

import numpy as np
import ml_dtypes

H = 2048
E = 8
TOPK = 2
P = 128
G = H // P  # 16

BF16 = ml_dtypes.bfloat16

_compiled_cache = {}
LAST_RESULTS = None  # test harness introspection


def _chunk_list(C):
    chunks = []
    off = 0
    while off < C:
        sz = min(512, C - off)
        chunks.append((off, sz))
        off += sz
    return chunks


def _build_nc(C, with_b1):
    # Wait-legality invariant (walrus allows ONE semaphore wait per
    # instruction, and Tile's wait emission is not transitive across
    # processors): every DMA must have a single dependency frontier.
    #   - w1/xt/w2 loads go to fresh slots (never reused), so their only
    #     wait is a queue ring wait.  w2 gets fresh slots by releasing the
    #     w1/xt pools (stack allocator, LIFO) and opening a new pool.
    #   - outputs are batched into 4 SWDGE DMAs on fresh lanes (DVE dep).
    import concourse.bass as bass
    import concourse.mybir as mybir
    from concourse import tile

    chunks = _chunk_list(C)
    NCH = len(chunks)
    f32 = mybir.dt.float32
    bf16 = mybir.dt.bfloat16
    RELU = mybir.ActivationFunctionType.Relu
    J = 4  # k-tiles per batched weight/xt DMA
    NJ = G // J  # 4 batched DMAs per tensor
    GY = 4  # g-tiles per batched output DMA

    nc = bass.Bass()
    xt_d = nc.dram_tensor("xt", [H, C], bf16, kind="ExternalInput")
    w1_d = nc.dram_tensor("w1", [H, H], bf16, kind="ExternalInput")
    w2_d = nc.dram_tensor("w2", [H, H], bf16, kind="ExternalInput")
    if with_b1:
        b1_d = nc.dram_tensor("b1r", [1, H], f32, kind="ExternalInput")
    yt_d = nc.dram_tensor("yt", [H, C], bf16, kind="ExternalOutput")

    with tile.TileContext(nc) as tc:
        with (
            tc.tile_pool(name="hp", bufs=G) as hp,
            tc.tile_pool(name="yp", bufs=G // GY) as yp,
            tc.tile_pool(name="bp", bufs=1) as bp,
            tc.tile_pool(name="pp", bufs=2, space="PSUM") as pp,
        ):
            if with_b1:
                b1_sb = bp.tile([1, H], f32, tag="b1")
                nc.sync.dma_start(b1_sb[:], b1_d[:])
                ones_sb = bp.tile([1, 512], f32, tag="ones")
                nc.vector.memset(ones_sb[:], 1.0)

            hts = []
            with (
                tc.tile_pool(name="wp1", bufs=NJ) as wp1,
                tc.tile_pool(name="xp", bufs=NJ) as xp,
            ):
                # batched loads: 4 DMAs each for w1 and xt (J k-tiles/DMA).
                # w1 first so w1_j sits on HWDGE queue j; w2_j is emitted at
                # HWDGE position 8+j -> same queue j, making its WAW vs the
                # w1_j load an implicit same-queue FIFO dependency.
                w1s, xts = [], []
                for j in range(NJ):
                    w1t = wp1.tile([P, J, H], bf16, tag="w", name=f"w1_{j}")
                    nc.sync.dma_start(
                        w1t[:],
                        w1_d[j * J * P:(j + 1) * J * P, :].rearrange(
                            "(kk p) h -> p kk h", p=P
                        ),
                    )
                    w1s.append(w1t)
                for j in range(NJ):
                    xtt = xp.tile([P, J, C], bf16, tag="x", name=f"xt_{j}")
                    nc.sync.dma_start(
                        xtt[:],
                        xt_d[j * J * P:(j + 1) * J * P, :].rearrange(
                            "(kk p) c -> p kk c", p=P
                        ),
                    )
                    xts.append(xtt)

                # phase 1: hT[g] = relu(sum_k W1[k,g].T @ XT[k] (+ b1[g]))
                for g in range(G):
                    pss = [
                        pp.tile(
                            [P, 512], f32, tag=f"ps{ci}", name=f"ps1_{g}_{ci}"
                        )
                        for ci in range(NCH)
                    ]
                    if with_b1:
                        for ci, (off, sz) in enumerate(chunks):
                            nc.tensor.matmul(
                                pss[ci][:, :sz],
                                b1_sb[:, g * P:(g + 1) * P],
                                ones_sb[:, :sz],
                                start=True,
                                stop=False,
                            )
                    for k in range(G):
                        for ci, (off, sz) in enumerate(chunks):
                            nc.tensor.matmul(
                                pss[ci][:, :sz],
                                w1s[k // J][:, k % J, g * P:(g + 1) * P],
                                xts[k // J][:, k % J, off:off + sz],
                                start=(k == 0 and not with_b1),
                                stop=(k == G - 1),
                            )
                    ht = hp.tile([P, C], bf16, tag="h", name=f"ht_{g}")
                    for ci, (off, sz) in enumerate(chunks):
                        nc.scalar.activation(
                            ht[:, off:off + sz], pss[ci][:, :sz], RELU
                        )
                    hts.append(ht)

                # w2_j reuses w1_j's slot (same tag, same alloc order) and
                # its HWDGE queue: the slot WAW collapses into queue FIFO
                # order, leaving only the single PE WAR wait.
                w2s = []
                for j in range(NJ):
                    w2t = wp1.tile([P, J, H], bf16, tag="w", name=f"w2_{j}")
                    nc.sync.dma_start(
                        w2t[:],
                        w2_d[j * J * P:(j + 1) * J * P, :].rearrange(
                            "(kk p) h -> p kk h", p=P
                        ),
                    )
                    w2s.append(w2t)

                # phase 2: yT[g] = sum_k W2[k,g].T @ hT[k].
                # Out-DMAs all land on SWDGE lane 0 (7 scrap dummies pad the
                # round-robin between them): same-lane FIFO ordering lets the
                # post-finalize pass strip their lane waits and lets the tail
                # drain keep a single wait covering every output.  The last
                # group is a single g so the final DMA tail is short.
                scrap_d = nc.dram_tensor("scrapd", [1, 64], bf16)
                scrap_sb = bp.tile([1, 64], bf16, tag="scrap")
                GYS = [6, 5, 4, 1]
                gy_off = 0
                for gy, gy_n in enumerate(GYS):
                    yt_sb = yp.tile(
                        [P, gy_n, C], bf16, tag="y", name=f"y_{gy}"
                    )
                    for gg in range(gy_n):
                        g = gy_off + gg
                        pss = [
                            pp.tile(
                                [P, 512],
                                f32,
                                tag=f"ps{ci}",
                                name=f"ps2_{g}_{ci}",
                            )
                            for ci in range(NCH)
                        ]
                        for k in range(G):
                            for ci, (off, sz) in enumerate(chunks):
                                nc.tensor.matmul(
                                    pss[ci][:, :sz],
                                    w2s[k // J][:, k % J, g * P:(g + 1) * P],
                                    hts[k][:, off:off + sz],
                                    start=(k == 0),
                                    stop=(k == G - 1),
                                )
                        for ci, (off, sz) in enumerate(chunks):
                            nc.vector.tensor_copy(
                                yt_sb[:, gg, off:off + sz], pss[ci][:, :sz]
                            )
                    nc.gpsimd.dma_start(
                        yt_d[gy_off * P:(gy_off + gy_n) * P, :].rearrange(
                            "(gg p) c -> p gg c", p=P
                        ),
                        yt_sb[:],
                    )
                    gy_off += gy_n
                    if gy < len(GYS) - 1:
                        for dpad in range(7):
                            nc.gpsimd.dma_start(
                                scrap_sb[0:1, gy * 7 + dpad:gy * 7 + dpad + 1],
                                scrap_d[0:1, gy * 7 + dpad:gy * 7 + dpad + 1],
                            )

    nc.finalize()

    # Post-finalize wait legalization: the NJ w2-load DMAs carry two waits
    # (PE slot-release WAR + same-queue w1-load WAW).  Hardware allows one
    # wait per DMA.  The WAW is provably redundant: every PE matmul that
    # read w1_j waited on its load DMA, so the PE wait transitively implies
    # load completion (and the w2 DMA shares the w1 DMA's FIFO queue).
    fixed_w2 = fixed_out = fixed_drain = 0
    for blk in nc.m.functions[0].blocks:
        for inst in blk.instructions:
            si = getattr(inst, "sync_info", None)
            if si is None or len(si.on_wait) < 2:
                continue
            tn = type(inst).__name__
            waits = {w.ant_name.rstrip("0123456789_44").rstrip("_"): w
                     for w in si.on_wait}
            kinds = sorted(w.ant_name for w in si.on_wait)
            if tn == "InstDMACopy" and len(si.on_wait) == 2:
                k0 = [w for w in si.on_wait if w.ant_name.startswith("PE")]
                k1 = [w for w in si.on_wait if w.ant_name.startswith("DVE")]
                if k0:
                    # w2 load: {PE WAR, same-queue w1-load WAW}; queue FIFO
                    # plus PE-transitivity make the DMAHW wait redundant.
                    assert any(
                        w.ant_name.startswith("DMAHW") for w in si.on_wait
                    ), f"{inst.name}: {kinds}"
                    si.on_wait = k0
                    inst.sync_info = si
                    fixed_w2 += 1
                    continue
                if k1:
                    # chained out-DMA: {DVE copies, same-lane predecessor};
                    # same-lane SWDGE FIFO makes the lane wait redundant.
                    assert any(
                        w.ant_name.startswith("DMASW") for w in si.on_wait
                    ), f"{inst.name}: {kinds}"
                    si.on_wait = k1
                    inst.sync_info = si
                    fixed_out += 1
                    continue
                raise AssertionError(f"unexpected 2-wait DMA {inst.name}: {kinds}")
            if tn == "InstDrain":
                # tail drain: keep only the lane-0 wait (covers all chained
                # out-DMAs via FIFO); engines are covered by the barrier
                # butterfly that follows, loads by their compute consumers.
                lane = [w for w in si.on_wait if w.ant_name.startswith("DMASW")]
                assert len(lane) >= 1, f"{inst.name}: {kinds}"
                lane = sorted(lane, key=lambda w: w.ant_name)[:1]
                si.on_wait = lane
                inst.sync_info = si
                fixed_drain += 1
                continue
            raise AssertionError(f"unexpected multi-wait {tn} {inst.name}: {kinds}")
    assert fixed_w2 == NJ, f"expected {NJ} w2-load fixups, found {fixed_w2}"
    assert fixed_out in (3, 4), f"expected 3-4 out-chain fixups, found {fixed_out}"
    assert fixed_drain == 1, f"expected 1 drain fixup, found {fixed_drain}"
    return nc


def _get_nc(C, with_b1):
    key = (C, with_b1)
    if key not in _compiled_cache:
        _compiled_cache[key] = _build_nc(C, with_b1)
    return _compiled_cache[key]


def _gate_host(x, gate_W, gate_b):
    """Gate softmax + top-2, bit-matching the jax fp32 reference on CPU."""
    try:
        import jax
        import jax.numpy as jnp

        cpu = jax.devices("cpu")[0]
        with jax.default_device(cpu):
            xs = jnp.asarray(x, device=cpu)
            gw = jnp.asarray(gate_W, device=cpu)
            gb = jnp.asarray(gate_b, device=cpu)
            scores = jax.nn.softmax(xs @ gw.T + gb, axis=-1)
            tv, ti = jax.lax.top_k(scores, TOPK)
            return np.asarray(scores), np.asarray(tv), np.asarray(ti)
    except Exception:
        logits = x.astype(np.float32) @ gate_W.T.astype(np.float32) + gate_b
        m = logits.max(axis=-1, keepdims=True)
        ex = np.exp(logits - m)
        scores = ex / ex.sum(axis=-1, keepdims=True)
        order = np.argsort(-scores, axis=1, kind="stable")
        ti = order[:, :TOPK]
        tv = np.take_along_axis(scores, ti, axis=1)
        return scores, tv, ti


def kernel(x, gate_W, gate_b, W1, b1, W2, b2):
    global LAST_RESULTS
    from concourse.bass_utils import run_bass_kernel_spmd
    import os

    x = np.ascontiguousarray(x, dtype=np.float32)
    n_tokens = x.shape[0]
    b1 = np.asarray(b1, dtype=np.float32)
    b2 = np.asarray(b2, dtype=np.float32)

    scores, tv, ti = _gate_host(x, gate_W, gate_b)

    rows_l, wts_l = [], []
    for e in range(E):
        sel = ti == e  # [N, 2]
        hit = sel.any(axis=1)
        rows = np.nonzero(hit)[0]
        we = np.where(sel[rows, 0], tv[rows, 0], tv[rows, 1])
        rows_l.append(rows)
        wts_l.append(we.astype(np.float32))

    maxc = max(len(r) for r in rows_l)
    C = max(64, ((maxc + 63) // 64) * 64)
    with_b1 = bool(np.any(b1))

    nc = _get_nc(C, with_b1)

    in_maps = []
    for e in range(E):
        rows = rows_l[e]
        XeT = np.zeros((H, C), dtype=BF16)
        XeT[:, : len(rows)] = x[rows].T.astype(BF16)
        im = {
            "xt": XeT,
            "w1": np.ascontiguousarray(W1[e]).astype(BF16),
            "w2": np.ascontiguousarray(W2[e]).astype(BF16),
        }
        if with_b1:
            im["b1r"] = np.ascontiguousarray(b1[e].reshape(1, H))
        in_maps.append(im)

    trace = bool(int(os.environ.get("MOE_TRACE", "0")))
    res = run_bass_kernel_spmd(
        nc,
        in_maps,
        list(range(E)),
        trace=trace,
        trace_cores=list(range(E)) if trace else None,
    )
    LAST_RESULTS = res

    out = np.zeros((n_tokens, H), dtype=np.float32)
    for e in range(E):
        rows = rows_l[e]
        yt = np.asarray(res.results[e]["yt"])  # [H, C] bf16
        ye = yt[:, : len(rows)].T.astype(np.float32)
        if b2 is not None and np.any(b2[e]):
            ye = ye + b2[e][None, :]
        out[rows] += wts_l[e][:, None] * ye

    return out, scores


# revision 14
# speedup vs baseline: 1.0214x; 1.0214x over previous
"""MoE (8 experts, top-2, H=2048, N=4096 tokens) Trainium2 kernel.

Strategy (expert-parallel over 8 NeuronCores):
  - Gate (softmax + top-2) computed host-side with jax-CPU fp32 so the
    routing decisions bit-match the jax reference.
  - Each core c handles expert e=c: tokens routed to e are gathered,
    transposed to X^T [H, C] (C = padded capacity), cast to bf16.
  - On-device per core: hT = relu(W1T.T @ XT [+ b1]); yT = second matmul.
    Weights stay stationary; both matmuls emit transposed activations so
    no on-device transpose is needed. bf16 inputs, fp32 PSUM accumulate.
  - b1 (if nonzero) enters as a rank-1 matmul into the psum group;
    b2 (if nonzero) is added host-side.
  - Host scatters y back: out[idx_e] += w_e[:,None] * y_e.

Hardware constraint honored throughout: each compute instruction carries at
most ONE semaphore wait, so no instruction may depend on two distinct
processors (e.g. PSUM-ready + DMA-loaded); staging tiles are never reused.
"""

import numpy as np
import ml_dtypes

H = 2048
E = 8
TOPK = 2
P = 128
G = H // P  # 16

BF16 = ml_dtypes.bfloat16

_compiled_cache = {}
LAST_RESULTS = None  # test harness introspection


def _chunk_list(C):
    chunks = []
    off = 0
    while off < C:
        sz = min(512, C - off)
        chunks.append((off, sz))
        off += sz
    return chunks


def _build_nc(C, with_b1):
    # Wait-legality invariant (walrus allows ONE semaphore wait per
    # instruction, and Tile's wait emission is not transitive across
    # processors): every DMA must have a single dependency frontier.
    #   - w1/xt/w2 loads go to fresh slots (never reused), so their only
    #     wait is a queue ring wait.  w2 gets fresh slots by releasing the
    #     w1/xt pools (stack allocator, LIFO) and opening a new pool.
    #   - outputs are batched into 4 SWDGE DMAs on fresh lanes (DVE dep).
    import concourse.bass as bass
    import concourse.mybir as mybir
    from concourse import tile

    chunks = _chunk_list(C)
    NCH = len(chunks)
    f32 = mybir.dt.float32
    bf16 = mybir.dt.bfloat16
    RELU = mybir.ActivationFunctionType.Relu
    J = 4  # k-tiles per batched weight/xt DMA
    NJ = G // J  # 4 batched DMAs per tensor
    GY = 4  # g-tiles per batched output DMA

    nc = bass.Bass()
    xt_d = nc.dram_tensor("xt", [H, C], bf16, kind="ExternalInput")
    w1_d = nc.dram_tensor("w1", [H, H], bf16, kind="ExternalInput")
    w2_d = nc.dram_tensor("w2", [H, H], bf16, kind="ExternalInput")
    if with_b1:
        b1_d = nc.dram_tensor("b1r", [1, H], f32, kind="ExternalInput")
    yt_d = nc.dram_tensor("yt", [H, C], bf16, kind="ExternalOutput")

    with tile.TileContext(nc) as tc:
        with (
            tc.tile_pool(name="hp", bufs=G) as hp,
            tc.tile_pool(name="yp", bufs=G // GY) as yp,
            tc.tile_pool(name="bp", bufs=1) as bp,
            tc.tile_pool(name="pp", bufs=2, space="PSUM") as pp,
        ):
            if with_b1:
                b1_sb = bp.tile([1, H], f32, tag="b1")
                nc.sync.dma_start(b1_sb[:], b1_d[:])
                ones_sb = bp.tile([1, 512], f32, tag="ones")
                nc.vector.memset(ones_sb[:], 1.0)

            hts = []
            with (
                tc.tile_pool(name="wp1", bufs=NJ + 1) as wp1,
                tc.tile_pool(name="xp", bufs=NJ) as xp,
            ):
                # batched loads: 4 DMAs each for w1 and xt (J k-tiles/DMA).
                # w1 first so w1_j sits on HWDGE queue j; w2_j is emitted at
                # HWDGE position 8+j -> same queue j, making its WAW vs the
                # w1_j load an implicit same-queue FIFO dependency.
                w1s, xts = [], []
                for j in range(NJ):
                    w1t = wp1.tile([P, J, H], bf16, tag="w", name=f"w1_{j}")
                    nc.sync.dma_start(
                        w1t[:],
                        w1_d[j * J * P:(j + 1) * J * P, :].rearrange(
                            "(kk p) h -> p kk h", p=P
                        ),
                    )
                    w1s.append(w1t)
                for j in range(NJ):
                    xtt = xp.tile([P, J, C], bf16, tag="x", name=f"xt_{j}")
                    nc.sync.dma_start(
                        xtt[:],
                        xt_d[j * J * P:(j + 1) * J * P, :].rearrange(
                            "(kk p) c -> p kk c", p=P
                        ),
                    )
                    xts.append(xtt)

                # phase 1: hT[g] = relu(sum_k W1[k,g].T @ XT[k] (+ b1[g]))
                for g in range(G):
                    pss = [
                        pp.tile(
                            [P, 512], f32, tag=f"ps{ci}", name=f"ps1_{g}_{ci}"
                        )
                        for ci in range(NCH)
                    ]
                    if with_b1:
                        for ci, (off, sz) in enumerate(chunks):
                            nc.tensor.matmul(
                                pss[ci][:, :sz],
                                b1_sb[:, g * P:(g + 1) * P],
                                ones_sb[:, :sz],
                                start=True,
                                stop=False,
                            )
                    for k in range(G):
                        for ci, (off, sz) in enumerate(chunks):
                            nc.tensor.matmul(
                                pss[ci][:, :sz],
                                w1s[k // J][:, k % J, g * P:(g + 1) * P],
                                xts[k // J][:, k % J, off:off + sz],
                                start=(k == 0 and not with_b1),
                                stop=(k == G - 1),
                            )
                    ht = hp.tile([P, C], bf16, tag="h", name=f"ht_{g}")
                    for ci, (off, sz) in enumerate(chunks):
                        nc.scalar.activation(
                            ht[:, off:off + sz], pss[ci][:, :sz], RELU
                        )
                    hts.append(ht)

                # w2_j reuses w1_j's slot (same tag, same alloc order) and
                # its HWDGE queue: the slot WAW collapses into queue FIFO
                # order, leaving only the single PE WAR wait.
                w2s = []
                for j in range(NJ):
                    w2t = wp1.tile([P, J, H], bf16, tag="w", name=f"w2_{j}")
                    nc.sync.dma_start(
                        w2t[:],
                        w2_d[j * J * P:(j + 1) * J * P, :].rearrange(
                            "(kk p) h -> p kk h", p=P
                        ),
                    )
                    w2s.append(w2t)

                # phase 2: yT[g] = sum_k W2[k,g].T @ hT[k].
                # Out-DMAs all land on SWDGE lane 0 (7 scrap dummies pad the
                # round-robin between them): same-lane FIFO ordering lets the
                # post-finalize pass strip their lane waits and lets the tail
                # drain keep a single wait covering every output.  The last
                # group is a single g so the final DMA tail is short.
                scrap_d = nc.dram_tensor("scrapd", [1, 64], bf16)
                scrap_sb = bp.tile([1, 64], bf16, tag="scrap")
                GYS = [6, 5, 4, 1]
                gy_off = 0
                for gy, gy_n in enumerate(GYS):
                    yt_sb = yp.tile(
                        [P, gy_n, C], bf16, tag="y", name=f"y_{gy}"
                    )
                    for gg in range(gy_n):
                        g = gy_off + gg
                        pss = [
                            pp.tile(
                                [P, 512],
                                f32,
                                tag=f"ps{ci}",
                                name=f"ps2_{g}_{ci}",
                            )
                            for ci in range(NCH)
                        ]
                        for k in range(G):
                            for ci, (off, sz) in enumerate(chunks):
                                nc.tensor.matmul(
                                    pss[ci][:, :sz],
                                    w2s[k // J][:, k % J, g * P:(g + 1) * P],
                                    hts[k][:, off:off + sz],
                                    start=(k == 0),
                                    stop=(k == G - 1),
                                )
                        for ci, (off, sz) in enumerate(chunks):
                            nc.vector.tensor_copy(
                                yt_sb[:, gg, off:off + sz], pss[ci][:, :sz]
                            )
                    nc.gpsimd.dma_start(
                        yt_d[gy_off * P:(gy_off + gy_n) * P, :].rearrange(
                            "(gg p) c -> p gg c", p=P
                        ),
                        yt_sb[:],
                    )
                    gy_off += gy_n
                    if gy < len(GYS) - 1:
                        for dpad in range(7):
                            nc.gpsimd.dma_start(
                                scrap_sb[0:1, gy * 7 + dpad:gy * 7 + dpad + 1],
                                scrap_d[0:1, gy * 7 + dpad:gy * 7 + dpad + 1],
                            )

    nc.finalize()

    # Post-finalize wait legalization: the NJ w2-load DMAs carry two waits
    # (PE slot-release WAR + same-queue w1-load WAW).  Hardware allows one
    # wait per DMA.  The WAW is provably redundant: every PE matmul that
    # read w1_j waited on its load DMA, so the PE wait transitively implies
    # load completion (and the w2 DMA shares the w1 DMA's FIFO queue).
    fixed_w2 = fixed_out = fixed_drain = 0
    for blk in nc.m.functions[0].blocks:
        for inst in blk.instructions:
            si = getattr(inst, "sync_info", None)
            if si is None or len(si.on_wait) < 2:
                continue
            tn = type(inst).__name__
            waits = {w.ant_name.rstrip("0123456789_44").rstrip("_"): w
                     for w in si.on_wait}
            kinds = sorted(w.ant_name for w in si.on_wait)
            if tn == "InstDMACopy" and len(si.on_wait) == 2:
                k0 = [w for w in si.on_wait if w.ant_name.startswith("PE")]
                k1 = [w for w in si.on_wait if w.ant_name.startswith("DVE")]
                if k0:
                    # w2 load: {PE WAR, same-queue w1-load WAW}; queue FIFO
                    # plus PE-transitivity make the DMAHW wait redundant.
                    assert any(
                        w.ant_name.startswith("DMAHW") for w in si.on_wait
                    ), f"{inst.name}: {kinds}"
                    si.on_wait = k0
                    inst.sync_info = si
                    fixed_w2 += 1
                    continue
                if k1:
                    # chained out-DMA: {DVE copies, same-lane predecessor};
                    # same-lane SWDGE FIFO makes the lane wait redundant.
                    assert any(
                        w.ant_name.startswith("DMASW") for w in si.on_wait
                    ), f"{inst.name}: {kinds}"
                    si.on_wait = k1
                    inst.sync_info = si
                    fixed_out += 1
                    continue
                raise AssertionError(f"unexpected 2-wait DMA {inst.name}: {kinds}")
            if tn == "InstDrain":
                # tail drain: keep only the lane-0 wait (covers all chained
                # out-DMAs via FIFO); engines are covered by the barrier
                # butterfly that follows, loads by their compute consumers.
                lane = [w for w in si.on_wait if w.ant_name.startswith("DMASW")]
                assert len(lane) >= 1, f"{inst.name}: {kinds}"
                lane = sorted(lane, key=lambda w: w.ant_name)[:1]
                si.on_wait = lane
                inst.sync_info = si
                fixed_drain += 1
                continue
            raise AssertionError(f"unexpected multi-wait {tn} {inst.name}: {kinds}")
    assert 0 < fixed_w2 <= NJ, f"expected <={NJ} w2-load fixups, found {fixed_w2}"
    assert fixed_out in (3, 4), f"expected 3-4 out-chain fixups, found {fixed_out}"
    assert fixed_drain == 1, f"expected 1 drain fixup, found {fixed_drain}"
    return nc


def _get_nc(C, with_b1):
    key = (C, with_b1)
    if key not in _compiled_cache:
        _compiled_cache[key] = _build_nc(C, with_b1)
    return _compiled_cache[key]


def _gate_host(x, gate_W, gate_b):
    """Gate softmax + top-2, bit-matching the jax fp32 reference on CPU."""
    try:
        import jax
        import jax.numpy as jnp

        cpu = jax.devices("cpu")[0]
        with jax.default_device(cpu):
            xs = jnp.asarray(x, device=cpu)
            gw = jnp.asarray(gate_W, device=cpu)
            gb = jnp.asarray(gate_b, device=cpu)
            scores = jax.nn.softmax(xs @ gw.T + gb, axis=-1)
            tv, ti = jax.lax.top_k(scores, TOPK)
            return np.asarray(scores), np.asarray(tv), np.asarray(ti)
    except Exception:
        logits = x.astype(np.float32) @ gate_W.T.astype(np.float32) + gate_b
        m = logits.max(axis=-1, keepdims=True)
        ex = np.exp(logits - m)
        scores = ex / ex.sum(axis=-1, keepdims=True)
        order = np.argsort(-scores, axis=1, kind="stable")
        ti = order[:, :TOPK]
        tv = np.take_along_axis(scores, ti, axis=1)
        return scores, tv, ti


def kernel(x, gate_W, gate_b, W1, b1, W2, b2):
    global LAST_RESULTS
    from concourse.bass_utils import run_bass_kernel_spmd
    import os

    x = np.ascontiguousarray(x, dtype=np.float32)
    n_tokens = x.shape[0]
    b1 = np.asarray(b1, dtype=np.float32)
    b2 = np.asarray(b2, dtype=np.float32)

    scores, tv, ti = _gate_host(x, gate_W, gate_b)

    rows_l, wts_l = [], []
    for e in range(E):
        sel = ti == e  # [N, 2]
        hit = sel.any(axis=1)
        rows = np.nonzero(hit)[0]
        we = np.where(sel[rows, 0], tv[rows, 0], tv[rows, 1])
        rows_l.append(rows)
        wts_l.append(we.astype(np.float32))

    maxc = max(len(r) for r in rows_l)
    C = max(64, ((maxc + 7) // 8) * 8)
    with_b1 = bool(np.any(b1))

    nc = _get_nc(C, with_b1)

    in_maps = []
    for e in range(E):
        rows = rows_l[e]
        XeT = np.zeros((H, C), dtype=BF16)
        XeT[:, : len(rows)] = x[rows].T.astype(BF16)
        im = {
            "xt": XeT,
            "w1": np.ascontiguousarray(W1[e]).astype(BF16),
            "w2": np.ascontiguousarray(W2[e]).astype(BF16),
        }
        if with_b1:
            im["b1r"] = np.ascontiguousarray(b1[e].reshape(1, H))
        in_maps.append(im)

    trace = bool(int(os.environ.get("MOE_TRACE", "0")))
    res = run_bass_kernel_spmd(
        nc,
        in_maps,
        list(range(E)),
        trace=trace,
        trace_cores=list(range(E)) if trace else None,
    )
    LAST_RESULTS = res

    out = np.zeros((n_tokens, H), dtype=np.float32)
    for e in range(E):
        rows = rows_l[e]
        yt = np.asarray(res.results[e]["yt"])  # [H, C] bf16
        ye = yt[:, : len(rows)].T.astype(np.float32)
        if b2 is not None and np.any(b2[e]):
            ye = ye + b2[e][None, :]
        out[rows] += wts_l[e][:, None] * ye

    return out, scores


# revision 15
# speedup vs baseline: 1.0628x; 1.0405x over previous
"""MoE (8 experts, top-2, H=2048, N=4096 tokens) Trainium2 kernel.

Strategy (expert-parallel over 8 NeuronCores):
  - Gate (softmax + top-2) computed host-side with jax-CPU fp32 so the
    routing decisions bit-match the jax reference.
  - Each core c handles expert e=c: tokens routed to e are gathered,
    transposed to X^T [H, C] (C = padded capacity), cast to bf16.
  - On-device per core: hT = relu(W1T.T @ XT [+ b1]); yT = second matmul.
    Weights stay stationary; both matmuls emit transposed activations so
    no on-device transpose is needed. bf16 inputs, fp32 PSUM accumulate.
  - b1 (if nonzero) enters as a rank-1 matmul into the psum group;
    b2 (if nonzero) is added host-side.
  - Host scatters y back: out[idx_e] += w_e[:,None] * y_e.

Hardware constraint honored throughout: each compute instruction carries at
most ONE semaphore wait, so no instruction may depend on two distinct
processors (e.g. PSUM-ready + DMA-loaded); staging tiles are never reused.
"""

import numpy as np
import ml_dtypes

H = 2048
E = 8
TOPK = 2
P = 128
G = H // P  # 16

BF16 = ml_dtypes.bfloat16

_compiled_cache = {}
LAST_RESULTS = None  # test harness introspection


def _chunk_list(C):
    chunks = []
    off = 0
    while off < C:
        sz = min(512, C - off)
        chunks.append((off, sz))
        off += sz
    return chunks


def _build_nc(C, with_b1):
    # Wait-legality invariant (walrus allows ONE semaphore wait per
    # instruction, and Tile's wait emission is not transitive across
    # processors): every DMA must have a single dependency frontier.
    #   - w1/xt/w2 loads go to fresh slots (never reused), so their only
    #     wait is a queue ring wait.  w2 gets fresh slots by releasing the
    #     w1/xt pools (stack allocator, LIFO) and opening a new pool.
    #   - outputs are batched into 4 SWDGE DMAs on fresh lanes (DVE dep).
    import concourse.bass as bass
    import concourse.mybir as mybir
    from concourse import tile

    chunks = _chunk_list(C)
    NCH = len(chunks)
    f32 = mybir.dt.float32
    bf16 = mybir.dt.bfloat16
    RELU = mybir.ActivationFunctionType.Relu
    J = 4  # k-tiles per batched weight/xt DMA
    NJ = G // J  # 4 batched DMAs per tensor
    GY = 4  # g-tiles per batched output DMA

    nc = bass.Bass()
    xt_d = nc.dram_tensor("xt", [H, C], bf16, kind="ExternalInput")
    w1_d = nc.dram_tensor("w1", [H, H], bf16, kind="ExternalInput")
    w2_d = nc.dram_tensor("w2", [H, H], bf16, kind="ExternalInput")
    if with_b1:
        b1_d = nc.dram_tensor("b1r", [1, H], f32, kind="ExternalInput")
    yt_d = nc.dram_tensor("yt", [H, C], bf16, kind="ExternalOutput")

    with tile.TileContext(nc) as tc:
        with (
            tc.tile_pool(name="hp", bufs=G) as hp,
            tc.tile_pool(name="yp", bufs=G // GY) as yp,
            tc.tile_pool(name="bp", bufs=1) as bp,
            tc.tile_pool(name="pp", bufs=2, space="PSUM") as pp,
        ):
            if with_b1:
                b1_sb = bp.tile([1, H], f32, tag="b1")
                nc.sync.dma_start(b1_sb[:], b1_d[:])
                ones_sb = bp.tile([1, 512], f32, tag="ones")
                nc.vector.memset(ones_sb[:], 1.0)

            hts = []
            with (
                tc.tile_pool(name="wp1", bufs=NJ + 1) as wp1,
                tc.tile_pool(name="xp", bufs=NJ) as xp,
            ):
                # batched loads: 4 DMAs each for w1 and xt (J k-tiles/DMA).
                # w1 first so w1_j sits on HWDGE queue j; w2_j is emitted at
                # HWDGE position 8+j -> same queue j, making its WAW vs the
                # w1_j load an implicit same-queue FIFO dependency.
                w1s, xts = [], []
                for j in range(NJ):
                    w1t = wp1.tile([P, J, H], bf16, tag="w", name=f"w1_{j}")
                    w1s.append(w1t)
                    xtt = xp.tile([P, J, C], bf16, tag="x", name=f"xt_{j}")
                    xts.append(xtt)
                # half-batch loads, interleaved: 16 DMAs spread 2-per-queue so
                # each queue carries ~1.5 MiB and phase-1 data lands ~10us
                # sooner than with one 2 MiB DMA per queue.
                HJ = J // 2
                for j in range(NJ):
                    for hh in range(2):
                        r0 = (j * J + hh * HJ) * P
                        nc.sync.dma_start(
                            w1s[j][:, hh * HJ:(hh + 1) * HJ, :],
                            w1_d[r0:r0 + HJ * P, :].rearrange(
                                "(kk p) h -> p kk h", p=P
                            ),
                        )
                        nc.sync.dma_start(
                            xts[j][:, hh * HJ:(hh + 1) * HJ, :],
                            xt_d[r0:r0 + HJ * P, :].rearrange(
                                "(kk p) c -> p kk c", p=P
                            ),
                        )

                # phase 1: hT[g] = relu(sum_k W1[k,g].T @ XT[k] (+ b1[g]))
                for g in range(G):
                    pss = [
                        pp.tile(
                            [P, 512], f32, tag=f"ps{ci}", name=f"ps1_{g}_{ci}"
                        )
                        for ci in range(NCH)
                    ]
                    if with_b1:
                        for ci, (off, sz) in enumerate(chunks):
                            nc.tensor.matmul(
                                pss[ci][:, :sz],
                                b1_sb[:, g * P:(g + 1) * P],
                                ones_sb[:, :sz],
                                start=True,
                                stop=False,
                            )
                    for k in range(G):
                        for ci, (off, sz) in enumerate(chunks):
                            nc.tensor.matmul(
                                pss[ci][:, :sz],
                                w1s[k // J][:, k % J, g * P:(g + 1) * P],
                                xts[k // J][:, k % J, off:off + sz],
                                start=(k == 0 and not with_b1),
                                stop=(k == G - 1),
                            )
                    ht = hp.tile([P, C], bf16, tag="h", name=f"ht_{g}")
                    for ci, (off, sz) in enumerate(chunks):
                        nc.scalar.activation(
                            ht[:, off:off + sz], pss[ci][:, :sz], RELU
                        )
                    hts.append(ht)

                # w2_j reuses w1_j's slot (same tag, same alloc order) and
                # its HWDGE queue: the slot WAW collapses into queue FIFO
                # order, leaving only the single PE WAR wait.
                w2s = []
                for j in range(NJ):
                    w2t = wp1.tile([P, J, H], bf16, tag="w", name=f"w2_{j}")
                    nc.sync.dma_start(
                        w2t[:],
                        w2_d[j * J * P:(j + 1) * J * P, :].rearrange(
                            "(kk p) h -> p kk h", p=P
                        ),
                    )
                    w2s.append(w2t)

                # phase 2: yT[g] = sum_k W2[k,g].T @ hT[k].
                # Out-DMAs all land on SWDGE lane 0 (7 scrap dummies pad the
                # round-robin between them): same-lane FIFO ordering lets the
                # post-finalize pass strip their lane waits and lets the tail
                # drain keep a single wait covering every output.  The last
                # group is a single g so the final DMA tail is short.
                scrap_d = nc.dram_tensor("scrapd", [1, 64], bf16)
                scrap_sb = bp.tile([1, 64], bf16, tag="scrap")
                GYS = [6, 5, 4, 1]
                gy_off = 0
                for gy, gy_n in enumerate(GYS):
                    yt_sb = yp.tile(
                        [P, gy_n, C], bf16, tag="y", name=f"y_{gy}"
                    )
                    for gg in range(gy_n):
                        g = gy_off + gg
                        pss = [
                            pp.tile(
                                [P, 512],
                                f32,
                                tag=f"ps{ci}",
                                name=f"ps2_{g}_{ci}",
                            )
                            for ci in range(NCH)
                        ]
                        for k in range(G):
                            for ci, (off, sz) in enumerate(chunks):
                                nc.tensor.matmul(
                                    pss[ci][:, :sz],
                                    w2s[k // J][:, k % J, g * P:(g + 1) * P],
                                    hts[k][:, off:off + sz],
                                    start=(k == 0),
                                    stop=(k == G - 1),
                                )
                        for ci, (off, sz) in enumerate(chunks):
                            nc.vector.tensor_copy(
                                yt_sb[:, gg, off:off + sz], pss[ci][:, :sz]
                            )
                    nc.gpsimd.dma_start(
                        yt_d[gy_off * P:(gy_off + gy_n) * P, :].rearrange(
                            "(gg p) c -> p gg c", p=P
                        ),
                        yt_sb[:],
                    )
                    gy_off += gy_n
                    if gy < len(GYS) - 1:
                        for dpad in range(7):
                            nc.gpsimd.dma_start(
                                scrap_sb[0:1, gy * 7 + dpad:gy * 7 + dpad + 1],
                                scrap_d[0:1, gy * 7 + dpad:gy * 7 + dpad + 1],
                            )

    nc.finalize()

    # Post-finalize wait legalization: the NJ w2-load DMAs carry two waits
    # (PE slot-release WAR + same-queue w1-load WAW).  Hardware allows one
    # wait per DMA.  The WAW is provably redundant: every PE matmul that
    # read w1_j waited on its load DMA, so the PE wait transitively implies
    # load completion (and the w2 DMA shares the w1 DMA's FIFO queue).
    fixed_w2 = fixed_out = fixed_drain = 0
    for blk in nc.m.functions[0].blocks:
        for inst in blk.instructions:
            si = getattr(inst, "sync_info", None)
            if si is None or len(si.on_wait) < 2:
                continue
            tn = type(inst).__name__
            waits = {w.ant_name.rstrip("0123456789_44").rstrip("_"): w
                     for w in si.on_wait}
            kinds = sorted(w.ant_name for w in si.on_wait)
            if tn == "InstDMACopy" and len(si.on_wait) >= 2:
                k0 = [w for w in si.on_wait if w.ant_name.startswith("PE")]
                k1 = [w for w in si.on_wait if w.ant_name.startswith("DVE")]
                if k0:
                    # w2 load: {PE WAR, same-queue w1-load WAW}; queue FIFO
                    # plus PE-transitivity make the DMAHW wait redundant.
                    assert any(
                        w.ant_name.startswith("DMAHW") for w in si.on_wait
                    ), f"{inst.name}: {kinds}"
                    si.on_wait = k0
                    inst.sync_info = si
                    fixed_w2 += 1
                    continue
                if k1:
                    # chained out-DMA: {DVE copies, same-lane predecessor};
                    # same-lane SWDGE FIFO makes the lane wait redundant.
                    assert any(
                        w.ant_name.startswith("DMASW") for w in si.on_wait
                    ), f"{inst.name}: {kinds}"
                    si.on_wait = k1
                    inst.sync_info = si
                    fixed_out += 1
                    continue
                raise AssertionError(f"unexpected 2-wait DMA {inst.name}: {kinds}")
            if tn == "InstDrain":
                # tail drain: keep only the lane-0 wait (covers all chained
                # out-DMAs via FIFO); engines are covered by the barrier
                # butterfly that follows, loads by their compute consumers.
                lane = [w for w in si.on_wait if w.ant_name.startswith("DMASW")]
                assert len(lane) >= 1, f"{inst.name}: {kinds}"
                lane = sorted(lane, key=lambda w: w.ant_name)[:1]
                si.on_wait = lane
                inst.sync_info = si
                fixed_drain += 1
                continue
            raise AssertionError(f"unexpected multi-wait {tn} {inst.name}: {kinds}")
    assert 0 < fixed_w2 <= NJ, f"expected <={NJ} w2-load fixups, found {fixed_w2}"
    assert fixed_out in (3, 4), f"expected 3-4 out-chain fixups, found {fixed_out}"
    assert fixed_drain == 1, f"expected 1 drain fixup, found {fixed_drain}"
    return nc


def _get_nc(C, with_b1):
    key = (C, with_b1)
    if key not in _compiled_cache:
        _compiled_cache[key] = _build_nc(C, with_b1)
    return _compiled_cache[key]


def _gate_host(x, gate_W, gate_b):
    """Gate softmax + top-2, bit-matching the jax fp32 reference on CPU."""
    try:
        import jax
        import jax.numpy as jnp

        cpu = jax.devices("cpu")[0]
        with jax.default_device(cpu):
            xs = jnp.asarray(x, device=cpu)
            gw = jnp.asarray(gate_W, device=cpu)
            gb = jnp.asarray(gate_b, device=cpu)
            scores = jax.nn.softmax(xs @ gw.T + gb, axis=-1)
            tv, ti = jax.lax.top_k(scores, TOPK)
            return np.asarray(scores), np.asarray(tv), np.asarray(ti)
    except Exception:
        logits = x.astype(np.float32) @ gate_W.T.astype(np.float32) + gate_b
        m = logits.max(axis=-1, keepdims=True)
        ex = np.exp(logits - m)
        scores = ex / ex.sum(axis=-1, keepdims=True)
        order = np.argsort(-scores, axis=1, kind="stable")
        ti = order[:, :TOPK]
        tv = np.take_along_axis(scores, ti, axis=1)
        return scores, tv, ti


def kernel(x, gate_W, gate_b, W1, b1, W2, b2):
    global LAST_RESULTS
    from concourse.bass_utils import run_bass_kernel_spmd
    import os

    x = np.ascontiguousarray(x, dtype=np.float32)
    n_tokens = x.shape[0]
    b1 = np.asarray(b1, dtype=np.float32)
    b2 = np.asarray(b2, dtype=np.float32)

    scores, tv, ti = _gate_host(x, gate_W, gate_b)

    rows_l, wts_l = [], []
    for e in range(E):
        sel = ti == e  # [N, 2]
        hit = sel.any(axis=1)
        rows = np.nonzero(hit)[0]
        we = np.where(sel[rows, 0], tv[rows, 0], tv[rows, 1])
        rows_l.append(rows)
        wts_l.append(we.astype(np.float32))

    maxc = max(len(r) for r in rows_l)
    C = max(64, ((maxc + 7) // 8) * 8)
    with_b1 = bool(np.any(b1))

    nc = _get_nc(C, with_b1)

    in_maps = []
    for e in range(E):
        rows = rows_l[e]
        XeT = np.zeros((H, C), dtype=BF16)
        XeT[:, : len(rows)] = x[rows].T.astype(BF16)
        im = {
            "xt": XeT,
            "w1": np.ascontiguousarray(W1[e]).astype(BF16),
            "w2": np.ascontiguousarray(W2[e]).astype(BF16),
        }
        if with_b1:
            im["b1r"] = np.ascontiguousarray(b1[e].reshape(1, H))
        in_maps.append(im)

    trace = bool(int(os.environ.get("MOE_TRACE", "0")))
    res = run_bass_kernel_spmd(
        nc,
        in_maps,
        list(range(E)),
        trace=trace,
        trace_cores=list(range(E)) if trace else None,
    )
    LAST_RESULTS = res

    out = np.zeros((n_tokens, H), dtype=np.float32)
    for e in range(E):
        rows = rows_l[e]
        yt = np.asarray(res.results[e]["yt"])  # [H, C] bf16
        ye = yt[:, : len(rows)].T.astype(np.float32)
        if b2 is not None and np.any(b2[e]):
            ye = ye + b2[e][None, :]
        out[rows] += wts_l[e][:, None] * ye

    return out, scores
